# revision 22
# baseline (speedup 1.0000x reference)
"""Trainium2 Bass kernel for nn_JointMamba: 4-direction Mamba scan + GLU conv.

Sharding: phase 1 runs the 8 independent (batch, direction) Mamba blocks one
per NeuronCore; phase 2 reshards the merged feature maps over (image, row-half)
and runs the 3x3 GLU conv, one shard per core. Host does only permutations
(scan_jego / merge_jego are pure index shuffles, done on uint16 views of bf16).

Both phases run through a cached jax.jit(shard_map) runner so repeat calls
skip retrace/relower and reuse the loaded NEFF.
"""
import sys
import numpy as np

try:
    import concourse.bass as bass  # noqa: F401
except ImportError:
    sys.path.insert(0, "/opt/trn_rl_repo")

import concourse.bass as bass
import concourse.bacc as bacc
import concourse.mybir as mybir
from concourse import tile

F32 = mybir.dt.float32
BF16 = mybir.dt.bfloat16
ALU = mybir.AluOpType
AF = mybir.ActivationFunctionType

B, C, H8, W8 = 2, 256, 64, 64
D_INNER, D_STATE, D_CONV, DT_RANK = 512, 16, 4, 16
L = (H8 // 2) * W8  # 2048
EPS = 1e-5

_cache = {}


def _bf16_dtype():
    import ml_dtypes
    return ml_dtypes.bfloat16


def _bf16(x):
    return np.asarray(x, dtype=_bf16_dtype())


# ---------------------------------------------------------------------------
# host-side permutations (pure data movement, on uint16 views)
# ---------------------------------------------------------------------------

def scan_jego_np(d0, d1):
    d2w = np.concatenate([d0, d1], 3)
    d2h = np.concatenate([d0, d1], 2)
    b, c = d0.shape[:2]
    x0 = d2w[:, :, ::2, ::2].reshape(b, c, -1)
    x1 = np.swapaxes(d2h, 2, 3)[:, :, 1::2, 1::2].reshape(b, c, -1)
    x2 = d2w[:, :, ::2, 1::2].reshape(b, c, -1)[:, :, ::-1]
    x3 = np.swapaxes(d2h, 2, 3)[:, :, ::2, 1::2].reshape(b, c, -1)[:, :, ::-1]
    return np.stack([x0, x1, x2, x3], 1)  # [B,4,C,L]


def merge_jego_np(ys, ori_h, ori_w):
    # ys: [B,4,C,L]; the "+" merges disjoint (even/odd row) positions, so it
    # is exact on uint16 views of bf16 data too.
    b, k, c, Lx = ys.shape
    H, W = ori_h // 2, ori_w // 2
    y2w = np.zeros((b, c, ori_h, 2 * ori_w), ys.dtype)
    y2h = np.zeros((b, c, 2 * ori_h, ori_w), ys.dtype)
    y2w[:, :, ::2, ::2] = ys[:, 0].reshape(b, c, H, 2 * W)
    y2h[:, :, 1::2, 1::2] = np.swapaxes(ys[:, 1].reshape(b, c, W, 2 * H), 2, 3)
    y2w[:, :, ::2, 1::2] = ys[:, 2][:, :, ::-1].reshape(b, c, H, 2 * W)
    y2h[:, :, 1::2, ::2] = np.swapaxes(ys[:, 3][:, :, ::-1].reshape(b, c, W, 2 * H), 2, 3)
    d0w, d1w = np.split(y2w, 2, axis=3)
    d0h, d1h = np.split(y2h, 2, axis=2)
    return d0w + d0h, d1w + d1h


# ---------------------------------------------------------------------------
# cached SPMD runner (mirrors bass2jax.run_bass_via_pjrt but builds the jitted
# callable once per program, so repeat calls skip retrace / NEFF reload)
# ---------------------------------------------------------------------------

class Runner:
    """Cached SPMD runner. Input tensors listed in `cached_names` are hashed
    and kept resident on device between calls (re-uploaded only when their
    bytes change); donated output buffers are created device-side."""

    def __init__(self, nc, n_cores=8, cached_names=()):
        import jax
        from jax.sharding import Mesh, PartitionSpec, NamedSharding
        from jax.experimental.shard_map import shard_map
        from concourse import bass2jax
        from concourse.bass2jax import _bass_exec_p, partition_id_tensor

        bass2jax.install_neuronx_cc_hook()
        assert nc.dbg_addr is None
        self.jax = jax
        self.n_cores = n_cores
        self.cached_names = set(cached_names)
        self._dev_cache = {}  # name -> (digest, device_array)

        partition_name = (nc.partition_id_tensor.name
                          if nc.partition_id_tensor else None)
        in_names, out_names, out_avals, zero_shapes = [], [], [], []
        for alloc in nc.m.functions[0].allocations:
            if not isinstance(alloc, mybir.MemoryLocationSet):
                continue
            name = alloc.memorylocations[0].name
            if alloc.kind == "ExternalInput":
                if name != partition_name:
                    in_names.append(name)
            elif alloc.kind == "ExternalOutput":
                out_names.append(name)
                shape = tuple(alloc.tensor_shape)
                dtype = mybir.dt.np(alloc.dtype)
                out_avals.append(jax.core.ShapedArray(shape, dtype))
                zero_shapes.append((shape, dtype))
        self.in_names, self.out_names = in_names, out_names
        self.out_avals, self.zero_shapes = out_avals, zero_shapes
        n_params = len(in_names)
        n_outs = len(out_avals)
        all_in_names = list(in_names) + list(out_names)
        if partition_name is not None:
            all_in_names.append(partition_name)
        donate = tuple(range(n_params, n_params + n_outs))

        def _body(*args):
            operands = list(args)
            if partition_name is not None:
                operands.append(partition_id_tensor())
            outs = _bass_exec_p.bind(
                *operands,
                out_avals=tuple(out_avals),
                in_names=tuple(all_in_names),
                out_names=tuple(out_names),
                lowering_input_output_aliases=(),
                sim_require_finite=True,
                sim_require_nnan=True,
                nc=nc,
            )
            return tuple(outs)

        devices = jax.devices()[:n_cores]
        mesh = Mesh(np.asarray(devices), ("core",))
        self.sharding = NamedSharding(mesh, PartitionSpec("core"))
        in_specs = (PartitionSpec("core"),) * (n_params + n_outs)
        out_specs = (PartitionSpec("core"),) * n_outs
        self.sharded = jax.jit(
            shard_map(_body, mesh=mesh, in_specs=in_specs,
                      out_specs=out_specs, check_rep=False),
            donate_argnums=donate, keep_unused=True,
        )

    def _arg(self, name, in_maps):
        import hashlib
        mats = [np.asarray(m[name]) for m in in_maps]
        if name not in self.cached_names:
            return np.concatenate(mats, axis=0)
        h = hashlib.blake2b(digest_size=16)
        for a in mats:
            h.update(np.ascontiguousarray(a).view(np.uint8).data)
        dg = h.digest()
        hit = self._dev_cache.get(name)
        if hit is not None and hit[0] == dg:
            return hit[1]
        arr = self.jax.device_put(np.concatenate(mats, axis=0), self.sharding)
        self._dev_cache[name] = (dg, arr)
        return arr

    def run_raw(self, in_maps, overrides=None):
        jnp = self.jax.numpy
        args = [overrides[name] if overrides and name in overrides
                else self._arg(name, in_maps) for name in self.in_names]
        zeros = [jnp.zeros((self.n_cores * s[0], *s[1:]), d,
                           device=self.sharding)
                 for s, d in self.zero_shapes]
        return self.sharded(*args, *zeros)

    def run(self, in_maps, overrides=None):
        out_arrs = self.run_raw(in_maps, overrides)
        outs = []
        for c in range(self.n_cores):
            outs.append({
                name: np.asarray(out_arrs[i]).reshape(
                    self.n_cores, *self.out_avals[i].shape)[c]
                for i, name in enumerate(self.out_names)
            })
        return outs


def make_runner(nc, n_cores=8, cached_names=()):
    return Runner(nc, n_cores, cached_names).run


# ---------------------------------------------------------------------------
# device-side merge_jego: a jitted gather keeps the phase-1 -> phase-2
# intermediate on device (saves ~17MB of tunnel round-trip per call)
# ---------------------------------------------------------------------------

def _merge_xla(outT_g, xp):
    """merge_jego + pad + per-core dpad slicing, as pure layout ops
    (reshape/stack/flip only -- no gather/scatter). xp = jnp or np."""
    ys = outT_g.reshape(2, 4, 2, 128, L).reshape(2, 4, C, L)  # [b,k,c,L]
    R0 = ys[:, 0].reshape(2, C, 32, 64)
    R2 = xp.flip(ys[:, 2], -1).reshape(2, C, 32, 64)
    T1 = xp.swapaxes(ys[:, 1].reshape(2, C, 32, 64), 2, 3)   # [2,C,64,32]
    T3 = xp.swapaxes(xp.flip(ys[:, 3], -1).reshape(2, C, 32, 64), 2, 3)
    E0 = xp.stack([R0[..., :32], R2[..., :32]], -1).reshape(2, C, 32, 64)
    E1 = xp.stack([R0[..., 32:], R2[..., 32:]], -1).reshape(2, C, 32, 64)
    O0 = xp.stack([T3[:, :, :32, :], T1[:, :, :32, :]], -1).reshape(2, C, 32, 64)
    O1 = xp.stack([T3[:, :, 32:, :], T1[:, :, 32:, :]], -1).reshape(2, C, 32, 64)
    d0 = xp.stack([E0, O0], 3).reshape(2, C, 64, 64)
    d1 = xp.stack([E1, O1], 3).reshape(2, C, 64, 64)
    Dfull = xp.concatenate([d0, d1], 0)                       # [4,C,64,64]
    Dpad = xp.pad(Dfull, ((0, 0), (0, 0), (1, 1), (1, 1)))
    slices = []
    for core in range(8):
        img, half = divmod(core, 2)
        r0 = half * 32
        slices.append(Dpad[img, :, r0:r0 + 34, :].reshape(2, 128, 34 * 66))
    return xp.stack(slices).reshape(16, 128, 34 * 66)


def get_merge_dev(sharding):
    if "mergejit" in _cache:
        return _cache["mergejit"]
    import jax
    import jax.numpy as jnp

    fn = jax.jit(lambda g: _merge_xla(g, jnp), out_shardings=sharding)
    _cache["mergejit"] = fn
    return fn


# ---------------------------------------------------------------------------
# phase 1: per-(b,k) Mamba block on one core
# layout: channel-major ([d, t]) throughout; selective scan uses the native
# DVE TensorTensorScan along the free (t) axis, full-L [128, 2048] tiles.
# ---------------------------------------------------------------------------

def build_phase1():
    nc = bacc.Bacc("TRN2", target_bir_lowering=False, debug=False, num_devices=8)
    xT = nc.dram_tensor("xT", [2, 128, L], BF16, kind="ExternalInput")
    nwb = nc.dram_tensor("nwb", [2, 128, 2], F32, kind="ExternalInput")
    inwT = nc.dram_tensor("inwT", [2, 128, 2 * D_INNER], BF16, kind="ExternalInput")
    convw = nc.dram_tensor("convw", [4, 128, D_CONV], F32, kind="ExternalInput")
    convb = nc.dram_tensor("convb", [4, 128, 1], F32, kind="ExternalInput")
    xprojT = nc.dram_tensor("xprojT", [4, 128, 48], BF16, kind="ExternalInput")
    dtwT = nc.dram_tensor("dtwT", [16, D_INNER], BF16, kind="ExternalInput")
    dtb = nc.dram_tensor("dtb", [4, 128, 1], F32, kind="ExternalInput")
    AT = nc.dram_tensor("AT", [4, 128, D_STATE], F32, kind="ExternalInput")
    Dpt = nc.dram_tensor("Dpt", [4, 128, 1], F32, kind="ExternalInput")
    outwT = nc.dram_tensor("outwT", [4, 128, C], BF16, kind="ExternalInput")
    ones1 = nc.dram_tensor("ones1", [1, 128], BF16, kind="ExternalInput")
    oneM = nc.dram_tensor("oneM", [128, 128], BF16, kind="ExternalInput")  # 1/256
    outT = nc.dram_tensor("outT", [2, 128, L], BF16, kind="ExternalOutput")

    with tile.TileContext(nc) as tc:
        with tc.tile_pool(name="wp", bufs=1) as wp, \
             tc.tile_pool(name="big", bufs=1) as big, \
             tc.tile_pool(name="bcp", bufs=2) as bcp, \
             tc.tile_pool(name="sc", bufs=2) as sc, \
             tc.tile_pool(name="sr", bufs=1) as sr:

            # ---- load inputs
            x_t = [wp.tile([128, L], BF16, name=f"x{i}") for i in range(2)]
            for i in range(2):
                nc.sync.dma_start(out=x_t[i][:], in_=xT[i])
            nwb_t = wp.tile([128, 4], F32, name="nwb_t")
            for i in range(2):
                nc.sync.dma_start(out=nwb_t[:, 2 * i:2 * i + 2], in_=nwb[i])
            inw_t = [wp.tile([128, 2 * D_INNER], BF16, name=f"inw{i}") for i in range(2)]
            for i in range(2):
                nc.sync.dma_start(out=inw_t[i][:], in_=inwT[i])
            convw_t = [wp.tile([128, D_CONV], F32, name=f"cw{i}") for i in range(4)]
            convb_t = [wp.tile([128, 1], F32, name=f"cb{i}") for i in range(4)]
            xproj_t = [wp.tile([128, 48], BF16, name=f"xp{i}") for i in range(4)]
            dtb_t = [wp.tile([128, 1], F32, name=f"dtb{i}") for i in range(4)]
            A_t = [wp.tile([128, D_STATE], F32, name=f"A{i}") for i in range(4)]
            Dp_t = [wp.tile([128, 1], F32, name=f"Dp{i}") for i in range(4)]
            outw_t = [wp.tile([128, C], BF16, name=f"ow{i}") for i in range(4)]
            for i in range(4):
                nc.sync.dma_start(out=convw_t[i][:], in_=convw[i])
                nc.sync.dma_start(out=convb_t[i][:], in_=convb[i])
                nc.sync.dma_start(out=xproj_t[i][:], in_=xprojT[i])
                nc.sync.dma_start(out=dtb_t[i][:], in_=dtb[i])
                nc.sync.dma_start(out=A_t[i][:], in_=AT[i])
                nc.sync.dma_start(out=Dp_t[i][:], in_=Dpt[i])
                nc.sync.dma_start(out=outw_t[i][:], in_=outwT[i])
            dtw_t = wp.tile([16, D_INNER], BF16, name="dtw_t")
            nc.sync.dma_start(out=dtw_t[:], in_=dtwT[:])
            eps_t = wp.tile([128, 1], F32, name="eps_t")
            nc.vector.memset(eps_t[:], EPS)
            ones_t = wp.tile([1, 128], BF16, name="ones_t")
            nc.sync.dma_start(out=ones_t[:], in_=ones1[:])
            oneM_t = wp.tile([128, 128], BF16, name="oneM_t")
            nc.sync.dma_start(out=oneM_t[:], in_=oneM[:])

            x_ln = [big.tile([128, L], BF16, name=f"xln{i}") for i in range(2)]
            xa_pad = [big.tile([128, 3 + L], BF16, name=f"xap{i}") for i in range(4)]
            sz = [big.tile([128, L], BF16, name=f"sz{i}") for i in range(4)]
            u_t = [big.tile([128, L], BF16, name=f"u{i}") for i in range(4)]
            dt_t = [big.tile([128, L], BF16, name=f"dt{i}") for i in range(4)]
            dtu_t = [big.tile([128, L], BF16, name=f"dtu{i}") for i in range(4)]
            y_acc = [big.tile([128, L], BF16, name=f"ya{i}") for i in range(4)]
            dbc_sb = big.tile([48, L], BF16, name="dbc_sb")  # dt_lr | B | C rows

            with tc.tile_pool(name="psA", bufs=2, space="PSUM") as psA:
                # ---- layernorm: stats via PE broadcast-mean matmuls
                sq = [sc.tile([128, L], BF16, name=f"sq{i}", tag="dA") for i in range(2)]
                for i in range(2):
                    nc.scalar.activation(out=sq[i][:], in_=x_t[i][:], func=AF.Square)
                mu_p = psA.tile([128, L], F32, name="mu_p", tag="mm")
                for ch in range(4):
                    s4 = slice(ch * 512, (ch + 1) * 512)
                    for i in range(2):
                        nc.tensor.matmul(mu_p[:, s4], lhsT=oneM_t[:], rhs=x_t[i][:, s4],
                                         start=(i == 0), stop=(i == 1))
                ss_p = psA.tile([128, L], F32, name="ss_p", tag="mm")
                for ch in range(4):
                    s4 = slice(ch * 512, (ch + 1) * 512)
                    for i in range(2):
                        nc.tensor.matmul(ss_p[:, s4], lhsT=oneM_t[:], rhs=sq[i][:, s4],
                                         start=(i == 0), stop=(i == 1))
                # cen = x - mu (into scratch; scaled + biased into x_ln below)
                cen = [sc.tile([128, L], BF16, name=f"cen{i}", tag="h") for i in range(2)]
                for i in range(2):
                    nc.vector.scalar_tensor_tensor(
                        out=cen[i][:], in0=mu_p[:], scalar=-1.0, in1=x_t[i][:],
                        op0=ALU.mult, op1=ALU.add)
                # var = E[x^2] - mu^2 ; inv = exp(-0.5*ln(var+eps)); stats live
                # in PSUM to save SBUF
                mu2 = sc.tile([128, L], BF16, name="mu2", tag="yn")
                nc.scalar.activation(out=mu2[:], in_=mu_p[:], func=AF.Square)
                var_p = psA.tile([128, L], F32, name="var_p", tag="mm")
                nc.vector.scalar_tensor_tensor(
                    out=var_p[:], in0=mu2[:], scalar=-1.0, in1=ss_p[:],
                    op0=ALU.mult, op1=ALU.add)
                lnv_p = psA.tile([128, L], F32, name="lnv_p", tag="mm")
                nc.scalar.activation(out=lnv_p[:], in_=var_p[:], func=AF.Ln, bias=eps_t[:])
                inv_p = psA.tile([128, L], F32, name="inv_p", tag="mm")
                nc.scalar.activation(out=inv_p[:], in_=lnv_p[:], func=AF.Exp, scale=-0.5)
                for i in range(2):
                    nc.vector.tensor_tensor(out=cen[i][:], in0=cen[i][:],
                                            in1=inv_p[:], op=ALU.mult)
                    nc.scalar.activation(out=x_ln[i][:], in_=cen[i][:],
                                         func=AF.Identity,
                                         scale=nwb_t[:, 2 * i:2 * i + 1],
                                         bias=nwb_t[:, 2 * i + 1:2 * i + 2])

                # ---- in-proj -> xa (padded, for conv) and silu(z)
                for i in range(4):
                    nc.vector.memset(xa_pad[i][:, 0:3], 0.0)
                for m in range(8):
                    p = psA.tile([128, L], F32, name="inp_p", tag="mm")
                    for ch in range(4):
                        s4 = slice(ch * 512, (ch + 1) * 512)
                        for i in range(2):
                            nc.tensor.matmul(p[:, s4],
                                             lhsT=inw_t[i][:, m * 128:(m + 1) * 128],
                                             rhs=x_ln[i][:, s4],
                                             start=(i == 0), stop=(i == 1))
                    if m < 4:
                        nc.scalar.activation(out=xa_pad[m][:, 3:3 + L], in_=p[:],
                                             func=AF.Copy)
                    else:
                        nc.scalar.activation(out=sz[m - 4][:], in_=p[:], func=AF.Silu)

                # ---- depthwise causal conv(4) + silu -> u
                for i in range(4):
                    acc = sc.tile([128, L], BF16, name="acc", tag="dBu")
                    nc.vector.tensor_scalar_mul(out=acc[:], in0=xa_pad[i][:, 0:L],
                                                scalar1=convw_t[i][:, 0:1])
                    for tap in range(1, 4):
                        nc.vector.scalar_tensor_tensor(
                            out=acc[:], in0=xa_pad[i][:, tap:tap + L],
                            scalar=convw_t[i][:, tap:tap + 1], in1=acc[:],
                            op0=ALU.mult, op1=ALU.add)
                    nc.scalar.activation(out=u_t[i][:], in_=acc[:], func=AF.Silu,
                                         bias=convb_t[i][:])

                # ---- xproj -> dt_lr, B, C rows
                dbc_p = psA.tile([48, L], F32, name="dbc_p", tag="mm")
                for ch in range(4):
                    s4 = slice(ch * 512, (ch + 1) * 512)
                    for i in range(4):
                        nc.tensor.matmul(dbc_p[:, s4], lhsT=xproj_t[i][:],
                                         rhs=u_t[i][:, s4],
                                         start=(i == 0), stop=(i == 3))
                nc.scalar.activation(out=dbc_sb[:], in_=dbc_p[:], func=AF.Copy)

                # ---- dt = softplus(dt_w @ dt_lr + dt_b) = ln(1+exp(.)); dtu = dt*u
                # (no softplus table in this build; exp+ln share one table set)
                for m in range(4):
                    p = psA.tile([128, L], F32, name="dt_p", tag="mm")
                    for ch in range(4):
                        s4 = slice(ch * 512, (ch + 1) * 512)
                        nc.tensor.matmul(p[:, s4],
                                         lhsT=dtw_t[:, m * 128:(m + 1) * 128],
                                         rhs=dbc_sb[0:16, s4], start=True, stop=True)
                    ex = sc.tile([128, L], BF16, name="ex", tag="dBu")
                    nc.scalar.activation(out=ex[:], in_=p[:], func=AF.Exp,
                                         bias=dtb_t[m][:])
                    nc.scalar.activation(out=dt_t[m][:], in_=ex[:], func=AF.Ln,
                                         bias=1.0)
                for m in range(4):
                    nc.vector.tensor_tensor(out=dtu_t[m][:], in0=dt_t[m][:],
                                            in1=u_t[m][:], op=ALU.mult)

            # ---- selective scan over the 16 states, full-L tiles
            with tc.tile_pool(name="psB", bufs=4, space="PSUM") as psB:
                for n in range(D_STATE):
                    # stage row n at partition 0 (DMA moves across partitions)
                    brow = sr.tile([1, L], BF16, name="brow", tag="brow")
                    nc.sync.dma_start(out=brow[:], in_=dbc_sb[16 + n:17 + n, :])
                    crow = sr.tile([1, L], BF16, name="crow", tag="crow")
                    nc.sync.dma_start(out=crow[:], in_=dbc_sb[32 + n:33 + n, :])
                    B_bc = bcp.tile([128, L], BF16, name="B_bc", tag="Bbc")
                    C_bc = bcp.tile([128, L], BF16, name="C_bc", tag="Cbc")
                    for half in range(2):
                        hs = slice(half * 1024, (half + 1) * 1024)
                        pB = psB.tile([128, 1024], F32, name="pB", tag="bc4")
                        pC = psB.tile([128, 1024], F32, name="pC", tag="bc4")
                        for q in range(2):
                            qs = slice(half * 1024 + q * 512, half * 1024 + (q + 1) * 512)
                            ql = slice(q * 512, (q + 1) * 512)
                            nc.tensor.matmul(pB[:, ql], lhsT=ones_t[:],
                                             rhs=brow[:, qs],
                                             start=True, stop=True)
                            nc.tensor.matmul(pC[:, ql], lhsT=ones_t[:],
                                             rhs=crow[:, qs],
                                             start=True, stop=True)
                        nc.scalar.activation(out=B_bc[:, hs], in_=pB[:], func=AF.Copy)
                        nc.scalar.activation(out=C_bc[:, hs], in_=pC[:], func=AF.Copy)
                    for dblk in range(4):
                        dA = sc.tile([128, L], BF16, name="dA", tag="dA")
                        nc.scalar.activation(out=dA[:], in_=dt_t[dblk][:],
                                             func=AF.Exp,
                                             scale=A_t[dblk][:, n:n + 1])
                        dBu = sc.tile([128, L], BF16, name="dBu", tag="dBu")
                        nc.vector.tensor_tensor(out=dBu[:], in0=dtu_t[dblk][:],
                                                in1=B_bc[:], op=ALU.mult)
                        h = sc.tile([128, L], BF16, name="h", tag="h")
                        nc.vector.tensor_tensor_scan(
                            out=h[:], data0=dA[:], data1=dBu[:],
                            initial=0.0, op0=ALU.mult, op1=ALU.add)
                        if n == 0:
                            nc.vector.tensor_tensor(out=y_acc[dblk][:], in0=h[:],
                                                    in1=C_bc[:], op=ALU.mult)
                        else:
                            yn = sc.tile([128, L], BF16, name="yn", tag="yn")
                            nc.vector.tensor_tensor(out=yn[:], in0=h[:],
                                                    in1=C_bc[:], op=ALU.mult)
                            nc.vector.tensor_tensor(out=y_acc[dblk][:],
                                                    in0=y_acc[dblk][:],
                                                    in1=yn[:], op=ALU.add)

            # ---- y = (y_acc + Dp*u) * silu(z), gated in place; out = out_w @ y + x
            for m in range(4):
                nc.vector.scalar_tensor_tensor(
                    out=y_acc[m][:], in0=u_t[m][:], scalar=Dp_t[m][:],
                    in1=y_acc[m][:], op0=ALU.mult, op1=ALU.add)
                nc.vector.tensor_tensor(out=y_acc[m][:], in0=y_acc[m][:],
                                        in1=sz[m][:], op=ALU.mult)
            with tc.tile_pool(name="psC", bufs=2, space="PSUM") as psC:
                for m in range(2):
                    p = psC.tile([128, L], F32, name="out_p", tag="mm")
                    for ch in range(4):
                        s4 = slice(ch * 512, (ch + 1) * 512)
                        for i in range(4):
                            nc.tensor.matmul(p[:, s4],
                                             lhsT=outw_t[i][:, m * 128:(m + 1) * 128],
                                             rhs=y_acc[i][:, s4],
                                             start=(i == 0), stop=(i == 3))
                    o = sc.tile([128, L], BF16, name="o", tag="dA")
                    nc.vector.tensor_tensor(out=o[:], in0=p[:], in1=x_t[m][:],
                                            op=ALU.add)
                    nc.sync.dma_start(out=outT[m], in_=o[:])
    nc.compile()
    return nc


def prep_phase1_inputs(inputs, xs_bf, core):
    b, k = divmod(core, 4)
    A = -np.exp(inputs['A_log'][k]).astype(np.float32)          # [512, 16]
    return {
        "xT": np.ascontiguousarray(xs_bf[b, k]).reshape(2, 128, L),
        "nwb": np.stack([inputs['norm_w'][k].reshape(2, 128),
                         inputs['norm_b'][k].reshape(2, 128)], 2).astype(np.float32),
        "inwT": _bf16(inputs['in_w'][k].T).reshape(2, 128, 2 * D_INNER),
        "convw": inputs['conv_w'][k][:, 0, :].reshape(4, 128, D_CONV).astype(np.float32),
        "convb": inputs['conv_b'][k].reshape(4, 128, 1).astype(np.float32),
        "xprojT": _bf16(inputs['xproj_w'][k].T).reshape(4, 128, 48),
        "dtwT": _bf16(inputs['dt_w'][k].T),
        "dtb": inputs['dt_b'][k].reshape(4, 128, 1).astype(np.float32),
        "AT": A.reshape(4, 128, D_STATE),
        "Dpt": inputs['Dp'][k].reshape(4, 128, 1).astype(np.float32),
        "outwT": _bf16(inputs['out_w'][k].T).reshape(4, 128, C),
        "ones1": _bf16(np.ones((1, 128))),
        "oneM": _bf16(np.full((128, 128), 1.0 / 256.0)),
    }


def run_phase1(inputs, xs_bf):
    if "p1" not in _cache:
        nc = build_phase1()
        _cache["p1"] = nc
        _cache["p1run"] = Runner(nc, 8, cached_names=(
            "nwb", "inwT", "convw", "convb", "xprojT", "dtwT", "dtb",
            "AT", "Dpt", "outwT", "ones1", "oneM"))
    ins = [prep_phase1_inputs(inputs, xs_bf, core) for core in range(8)]
    return _cache["p1run"].run_raw(ins)[0]  # device array [16, 128, L] bf16


# ---------------------------------------------------------------------------
# phase 2: 3x3 conv + GLU, sharded over (image, row-half); bf16 input, so the
# only precision compensation needed is the weight low-order term.
# ---------------------------------------------------------------------------

def build_phase2():
    """Per core: dpad [2,128,34*66] bf16, wc/wlo [9,2,128,512] bf16 (lhsT per
    tap; wlo = error-compensation), bias [128,4] f32.
    Output o [2,128,2048] bf16 (= [256, 32, 64] GLU'd rows)."""
    nc = bacc.Bacc("TRN2", target_bir_lowering=False, debug=False, num_devices=8)
    dpad = nc.dram_tensor("dpad", [2, 128, 34 * 66], BF16, kind="ExternalInput")
    wc = nc.dram_tensor("wc", [9, 2, 128, 512], BF16, kind="ExternalInput")
    wlo = nc.dram_tensor("wlo", [9, 2, 128, 512], BF16, kind="ExternalInput")
    bias = nc.dram_tensor("bias", [128, 4], F32, kind="ExternalInput")
    out = nc.dram_tensor("o", [2, 128, 2048], BF16, kind="ExternalOutput")

    with tile.TileContext(nc) as tc:
        with tc.tile_pool(name="cw", bufs=1) as cw, \
             tc.tile_pool(name="cd", bufs=1) as cd, \
             tc.tile_pool(name="cpsum", bufs=2, space="PSUM") as cpsum, \
             tc.tile_pool(name="cact", bufs=3) as cact:
            dt_ = []
            for kc in range(2):
                d = cd.tile([128, 34 * 66], BF16, name=f"d{kc}")
                nc.sync.dma_start(out=d[:], in_=dpad[kc])
                dt_.append(d)
            wt = []
            wlo_t = []
            for tap in range(9):
                row_w = []
                row_l = []
                for kc in range(2):
                    w_ = cw.tile([128, 512], BF16, name=f"w{tap}_{kc}")
                    nc.sync.dma_start(out=w_[:], in_=wc[tap, kc])
                    row_w.append(w_)
                    wl_ = cw.tile([128, 512], BF16, name=f"wl{tap}_{kc}")
                    nc.sync.dma_start(out=wl_[:], in_=wlo[tap, kc])
                    row_l.append(wl_)
                wt.append(row_w)
                wlo_t.append(row_l)
            bias_t = cw.tile([128, 4], F32, name="bias_t")
            nc.sync.dma_start(out=bias_t[:], in_=bias[:])

            for rg in range(4):  # row groups of 8 output rows
                ps = []
                for m in range(4):  # co tiles of 128
                    p = cpsum.tile([128, 512], F32, name=f"ps{m}")
                    ps.append(p)
                    first = True
                    for tap in range(9):
                        dy, dx = divmod(tap, 3)
                        for kc in range(2):
                            rhs = dt_[kc][:, (rg * 8 + dy) * 66 + dx:]
                            rhs = bass.AP(rhs.tensor, rhs.offset,
                                          [rhs.ap[0], [66, 8], [1, 64]])
                            nc.tensor.matmul(
                                p[:], lhsT=wt[tap][kc][:, m * 128:(m + 1) * 128],
                                rhs=rhs, start=first, stop=False)
                            first = False
                            last = (tap == 8 and kc == 1)
                            nc.tensor.matmul(
                                p[:], lhsT=wlo_t[tap][kc][:, m * 128:(m + 1) * 128],
                                rhs=rhs, start=False, stop=last)
                # GLU: a = ps[0..1], g = ps[2..3]
                for m in range(2):
                    sg = cact.tile([128, 512], F32, name="sg")
                    nc.scalar.activation(out=sg[:], in_=ps[2 + m][:],
                                         func=AF.Sigmoid, bias=bias_t[:, 2 + m:3 + m])
                    av = cact.tile([128, 512], F32, name="av")
                    nc.scalar.activation(out=av[:], in_=ps[m][:],
                                         func=AF.Identity, bias=bias_t[:, m:m + 1])
                    og = cact.tile([128, 512], BF16, name="og")
                    nc.vector.tensor_tensor(out=og[:], in0=av[:], in1=sg[:], op=ALU.mult)
                    nc.sync.dma_start(out=out[m, :, rg * 512:(rg + 1) * 512], in_=og[:])
    nc.compile()
    return nc


def prep_phase2_weights(glu_w, glu_b):
    # wc[tap, kc, ci, co] = glu_w[co, kc*128+ci, dy, dx]
    w = np.transpose(glu_w, (2, 3, 1, 0)).reshape(9, 2, 128, 512)
    w_hi = _bf16(w)
    w_lo = _bf16(w - np.asarray(w_hi, np.float32))
    bias = glu_b.reshape(4, 128).T.copy().astype(np.float32)  # [128, 4]
    return w_hi, w_lo, bias


def run_phase2(dpad_dev, glu_w, glu_b):
    """dpad_dev: device array [16, 128, 34*66] bf16 (per-core dpad slices).
    Returns [4, 256, 64, 64] bf16 after conv+GLU."""
    if "p2" not in _cache:
        nc = build_phase2()
        _cache["p2"] = nc
        _cache["p2run"] = Runner(nc, 8, cached_names=("wc", "wlo", "bias"))
    w_hi, w_lo, bias = prep_phase2_weights(glu_w, glu_b)
    ins = [{"wc": w_hi, "wlo": w_lo, "bias": bias} for _ in range(8)]
    res = _cache["p2run"].run(ins, overrides={"dpad": dpad_dev})
    outf = np.zeros((4, 256, 64, 64), np.uint16)
    for core in range(8):
        img, half = divmod(core, 2)
        o = res[core]["o"].view(np.uint16).reshape(256, 32, 64)
        outf[img, :, half * 32:half * 32 + 32, :] = o
    return outf.view(_bf16_dtype())


# ---------------------------------------------------------------------------
# top level
# ---------------------------------------------------------------------------

def kernel(**inputs):
    inputs = {k: np.asarray(v) for k, v in inputs.items()}
    f0 = _bf16(np.asarray(inputs['feat0'], np.float32))
    f1 = _bf16(np.asarray(inputs['feat1'], np.float32))
    xs_u = scan_jego_np(f0.view(np.uint16), f1.view(np.uint16))  # [B,4,C,L] u16
    xs_bf = xs_u.view(_bf16_dtype())
    outT_dev = run_phase1(inputs, xs_bf)
    try:
        dpad_dev = get_merge_dev(_cache["p1run"].sharding)(outT_dev)
    except Exception:
        outT_np = np.asarray(outT_dev).view(np.uint16)
        dpad_dev = _merge_xla(outT_np, np).view(_bf16_dtype())
    desc = run_phase2(dpad_dev, np.asarray(inputs['glu_w'], np.float32),
                      np.asarray(inputs['glu_b'], np.float32))
    desc = np.asarray(desc, np.float32)
    dd0, dd1 = desc[:B], desc[B:]
    return np.stack([dd0.reshape(B, C, -1), dd1.reshape(B, C, -1)], 0)
